# revision 6
# baseline (speedup 1.0000x reference)
"""BlockStackingSGN kernel for 8 Trainium2 NeuronCores.

Data-parallel over batch B=4096 (512 rows/core; batch in the free dim,
hidden on partitions). Key optimizations over a bf16 tiling:

- fp8e4m3 DoubleRow matmuls for every 256-deep contraction: one PE
  instruction contracts both 128-row k-tiles in the cycles of one,
  halving PE time.
- The linear object-encoder output layer (no relu) is folded on the host
  into its four downstream consumers (AonB-left/right, clear, ontable
  first layers), deleting that layer's matmuls and evacuations.
- Power-of-2 scaling (weights x16) keeps fp8 weights out of the
  subnormal range; scales flow through relu/add transparently and are
  absorbed for free by activation-engine scale or a tensor_scalar
  multiply, so every PSUM evacuation is a single instruction.
- Early phases run two 256-wide layers per 4-bank PSUM tile so one
  evacuation instruction drains four matmul accumulations (GpSimd
  cannot read PSUM, so evacuations are split across Scalar+Vector only;
  GpSimd handles the SBUF-side pair adds and relu casts).
- All 80 output heads (AonB pairs / clear / ontable) accumulate into one
  PSUM bank via one-hot fp8 stationaries sliced from a sliding window;
  a single batched Sigmoid finishes the kernel.
"""

import sys

import numpy as np

sys.path.insert(0, "/opt/trn_rl_repo")

import concourse.bacc as bacc
import concourse.mybir as mybir
import concourse.tile as tile
from concourse.bass_utils import run_bass_kernel_spmd

dt = mybir.dt
AF = mybir.ActivationFunctionType
ALU = mybir.AluOpType
PM = mybir.MatmulPerfMode

N = 8
H = 256
B = 4096
IN = 3 * N
NCORES = 8
BC = B // NCORES          # 512 batch rows per core
W = BC
R = N * (N + 2)           # 80 output rows
S = 16.0                  # weight scale 2^4

F32 = dt.float32
BF16 = dt.bfloat16
FP8 = dt.float8e4

_CACHE = {}


def _wb_layout():
    """fp8 weight tile entries of [128, 2, 256] (512 cols each), ordered by
    first use (doubles as DMA arrival order)."""
    keys = []
    for n in range(N):
        keys.append(("oW1", n))
    for n in range(N):
        keys.append(("Wl", n))
        keys.append(("Wr", n))
    for n in range(N):
        keys.append(("Wc", n))
        keys.append(("Wt", n))
    keys += [("cW1",), ("tW1",), ("w2c",), ("w2t",), ("aW1",), ("w2a",)]
    return {k: i for i, k in enumerate(keys)}, len(keys)


WB_ENT, WB_N = _wb_layout()
WB_COLS = WB_N * 512


def _bias_layout():
    keys = []
    for n in range(N):
        for nm in ("b0", "b1", "bl", "br", "bc", "bt"):
            for m in range(2):
                keys.append((nm, n, m))
    for nm in ("cb1", "tb1", "ab1"):
        for m in range(2):
            keys.append((nm, m))
    keys.append(("finb",))
    return {k: i for i, k in enumerate(keys)}, len(keys)


BIAS_OFF, BIAS_COLS = _bias_layout()

N_DMA_CHUNKS = 8

# engine schedules per op kind (tunable): A=scalar, D=vector, P=gpsimd
SEQ_EARLY = "ADAD"        # 4-bank evacs in phases A-C
SEQ_Y1 = "DADA"           # y1 evacs
SEQ_RC = "PDPAPDPA"       # relu-cast phs->ph (SBUF, Pool allowed)
SEQ_ADD = "DDPDDP"        # pair adds (SBUF, Pool allowed)
SEQ_YE = "ADAD"           # pair y evacs


def _build(zero_bias):
    nc = bacc.Bacc("TRN2", target_bir_lowering=False, debug=False, num_devices=NCORES)

    d_xw0 = nc.dram_tensor("xw0", [IN, BC + N * H], BF16, kind="ExternalInput")
    d_wb = nc.dram_tensor("wb", [128, WB_COLS], FP8, kind="ExternalInput")
    d_bias = nc.dram_tensor("bias", [128, BIAS_COLS], F32, kind="ExternalInput")
    d_out = nc.dram_tensor("outT", [R, BC], F32, kind="ExternalOutput")

    with tile.TileContext(nc) as tc:
        with (
            tc.tile_pool(name="w", bufs=1) as wp,
            tc.tile_pool(name="act", bufs=1) as acp,
            tc.tile_pool(name="wk", bufs=4) as wk,
            tc.tile_pool(name="ph2", bufs=3) as php,
        ):
            xw0 = wp.tile([IN, BC + N * H], BF16, tag="xw0")
            nc.sync.dma_start(xw0[:], d_xw0[:])
            xT = xw0[:, :BC]
            bias = wp.tile([128, BIAS_COLS], F32, tag="bias")
            nc.gpsimd.dma_start(bias[:], d_bias[:])

            wb = wp.tile([128, WB_N, 2, 256], FP8, tag="wb")
            chunk = (WB_N + N_DMA_CHUNKS - 1) // N_DMA_CHUNKS
            for c in range(N_DMA_CHUNKS):
                eng = nc.gpsimd if c % 2 == 0 else nc.sync
                lo, hi = c * chunk, min((c + 1) * chunk, WB_N)
                if lo < hi:
                    eng.dma_start(wb[:, lo:hi], d_wb[:, lo * 512:hi * 512])

            def wsl(key, m):
                return wb[:, WB_ENT[key], :, m * 128:(m + 1) * 128]

            def w2sl(key, r):
                return wb[:, WB_ENT[key], :, 128 - r:256 - r]

            def bcol(key):
                return bias[:, BIAS_OFF[key]:BIAS_OFF[key] + 1]

            def engine(e):
                return {"A": nc.scalar, "D": nc.vector, "P": nc.gpsimd}[e]

            def subaps(nd):
                """iterate [128, W] sub-APs of a [128, ..., W] AP in order"""
                if len(nd.shape) == 2:
                    yield nd
                elif len(nd.shape) == 3:
                    for m in range(nd.shape[1]):
                        yield nd[:, m]
                else:
                    for q in range(nd.shape[1]):
                        for m in range(nd.shape[2]):
                            yield nd[:, q, m]

            def evac(e, out_nd, ps_nd, bkeys, relu, scale):
                """out = func(scale * psum + scale*bias); one instruction when
                biases are zero, else one per [128, W] sub-tile."""
                if zero_bias:
                    if e == "A":
                        func = AF.Relu if relu else AF.Identity
                        nc.scalar.activation(out_nd, ps_nd, func, scale=scale)
                    else:
                        en = engine(e)
                        if relu and scale == 1.0:
                            en.tensor_scalar(out_nd, ps_nd, 0.0, None, ALU.max)
                        elif relu:
                            en.tensor_scalar(out_nd, ps_nd, 0.0, scale,
                                             ALU.max, ALU.mult)
                        else:
                            en.tensor_scalar(out_nd, ps_nd, scale, None, ALU.mult)
                else:
                    for (o, p), bk in zip(zip(subaps(out_nd), subaps(ps_nd)), bkeys):
                        b = bcol(bk)
                        if e == "A" or (relu and scale != 1.0):
                            func = AF.Relu if relu else AF.Identity
                            nc.scalar.activation(o, p, func, bias=b, scale=scale)
                        elif relu:
                            engine(e).tensor_scalar(o, p, b, 0.0, ALU.add, ALU.max)
                        else:
                            engine(e).tensor_scalar(o, p, b, scale,
                                                    ALU.add, ALU.mult)

            cts = {}

            def pick(seq, key):
                c = cts.setdefault(key, [0])
                e = seq[c[0] % len(seq)]
                c[0] += 1
                return e

            # ================= era A: 4-bank psum tiles =================
            h0 = acp.tile([128, N, 2, W], FP8, tag="h0")
            h1 = acp.tile([128, N, 2, W], FP8, tag="h1")
            al = acp.tile([128, N, 2, W], BF16, tag="al")
            ar = acp.tile([128, N, 2, W], BF16, tag="ar")
            with tc.tile_pool(name="pa", bufs=2, space="PSUM") as pa:
                # L0 (bf16, contraction 24): two blocks per psum tile
                for n in range(0, N, 2):
                    pst = pa.tile([128, 2, 2, W], F32, tag="pa", name=f"psA{n}")
                    for q in range(2):
                        for m in range(2):
                            o = BC + (n + q) * H + m * 128
                            nc.tensor.matmul(pst[:, q, m], xw0[:, o:o + 128], xT,
                                             start=True, stop=True,
                                             skip_group_check=True)
                    evac(pick(SEQ_EARLY, "h0"), h0[:, n:n + 2], pst[:],
                         [("b0", n + q, m) for q in range(2) for m in range(2)],
                         True, 1.0)
                # L1 (DR)
                for n in range(0, N, 2):
                    pst = pa.tile([128, 2, 2, W], F32, tag="pa", name=f"psB{n}")
                    for q in range(2):
                        for m in range(2):
                            nc.tensor.matmul(pst[:, q, m], wsl(("oW1", n + q), m),
                                             h0[:, n + q], start=True, stop=True,
                                             perf_mode=PM.DoubleRow,
                                             skip_group_check=True)
                    evac(pick(SEQ_EARLY, "h1"), h1[:, n:n + 2], pst[:],
                         [("b1", n + q, m) for q in range(2) for m in range(2)],
                         True, 1.0)
                # al / ar (DR, enc folded; bf16 out at 2^8)
                for n in range(0, N, 2):
                    for dst, key, bk in ((al, "Wl", "bl"), (ar, "Wr", "br")):
                        pst = pa.tile([128, 2, 2, W], F32, tag="pa",
                                      name=f"psC{n}{key}")
                        for q in range(2):
                            for m in range(2):
                                nc.tensor.matmul(pst[:, q, m], wsl((key, n + q), m),
                                                 h1[:, n + q], start=True, stop=True,
                                                 perf_mode=PM.DoubleRow,
                                                 skip_group_check=True)
                        evac(pick(SEQ_EARLY, "al"), dst[:, n:n + 2], pst[:],
                             [(bk, n + q, m) for q in range(2) for m in range(2)],
                             False, 1.0 / S)

            # ================= era B: 2-bank psum + fin =================
            with (
                tc.tile_pool(name="pw", bufs=3, space="PSUM") as pw,
                tc.tile_pool(name="pf", bufs=1, space="PSUM") as pf,
            ):
                fin = pf.tile([128, BC], F32, tag="fin")
                n_fin = N * N + 2 * N
                fin_ct = [0]

                def fin_mm(w2key, r, rhs):
                    first = fin_ct[0] == 0
                    fin_ct[0] += 1
                    last = fin_ct[0] == n_fin
                    nc.tensor.matmul(fin[:], w2sl(w2key, r), rhs,
                                     start=first, stop=last,
                                     perf_mode=PM.DoubleRow)

                def dr2(ps3, key, rhs):
                    for m in range(2):
                        nc.tensor.matmul(ps3[:, m], wsl(key, m), rhs,
                                         start=True, stop=True,
                                         perf_mode=PM.DoubleRow,
                                         skip_group_check=True)

                def pred_thunk(n, w0k, w1k, b0k, b1k, w2k, r):
                    def go():
                        y0 = wk.tile([128, 2, W], FP8, tag="y0",
                                     name=f"y0_{n}_{w0k}")
                        pst = pw.tile([128, 2, W], F32, tag="pw",
                                      name=f"psY0{n}{w0k}")
                        dr2(pst, (w0k, n), h1[:, n])
                        evac("A", y0[:], pst[:],
                             [(b0k, n, m) for m in range(2)], True, 1.0 / (S * S))
                        y1 = wk.tile([128, 2, W], FP8, tag="y1",
                                     name=f"y1_{n}_{w0k}")
                        pst2 = pw.tile([128, 2, W], F32, tag="pw",
                                       name=f"psY1{n}{w0k}")
                        dr2(pst2, w1k, y0[:])
                        evac(pick(SEQ_Y1, "y1"), y1[:], pst2[:],
                             [(b1k, m) for m in range(2)], True, 1.0)
                        fin_mm(w2k, r, y1[:])
                    return go

                preds = []
                for n in range(N):
                    preds.append(pred_thunk(n, "Wc", ("cW1",), "bc", "cb1",
                                            ("w2c",), n * 10 + 8))
                    preds.append(pred_thunk(n, "Wt", ("tW1",), "bt", "tb1",
                                            ("w2t",), n * 10 + 9))

                # pair loop: 2-pair blocks, preds interleaved
                for i in range(N):
                    for jj in range(0, N, 2):
                        phs = php.tile([128, 2, 2, W], BF16, tag="phs",
                                       name=f"phs{i}{jj}")
                        for u in range(2):
                            ea = pick(SEQ_ADD, "add")
                            engine(ea).tensor_tensor(phs[:, u], al[:, i],
                                                     ar[:, jj + u], ALU.add)
                        ph = php.tile([128, 2, 2, W], FP8, tag="ph",
                                      name=f"ph{i}{jj}")
                        e = pick(SEQ_RC, "rc")
                        if e == "A":
                            nc.scalar.activation(ph[:], phs[:], AF.Relu,
                                                 scale=1.0 / S)
                        else:
                            engine(e).tensor_scalar(ph[:], phs[:], 0.0, 1.0 / S,
                                                    ALU.max, ALU.mult)
                        if jj % 4 == 0 and preds:
                            preds.pop(0)()
                        for u in range(2):
                            j = jj + u
                            pst = pw.tile([128, 2, W], F32, tag="pw",
                                          name=f"psP{i}{j}")
                            dr2(pst, ("aW1",), ph[:, u])
                            y = wk.tile([128, 2, W], FP8, tag="y", name=f"y{i}{j}")
                            evac(pick(SEQ_YE, "ye"), y[:], pst[:],
                                 [("ab1", m) for m in range(2)], True, 1.0)
                            fin_mm(("w2a",), i * 10 + j, y[:])
                for t in preds:
                    t()
                assert fin_ct[0] == n_fin

                # batched sigmoid + store
                outT = wk.tile([128, BC], F32, tag="outT")
                nc.scalar.activation(outT[:], fin[:], AF.Sigmoid,
                                     bias=bcol(("finb",)), scale=1.0 / (S ** 3))
                nc.sync.dma_start(d_out[:], outT[:R, :])

    nc.compile()
    return nc


def _prep_inputs(inputs):
    import ml_dtypes

    bf = ml_dtypes.bfloat16
    f8 = ml_dtypes.float8_e4m3fn
    f32a = lambda a: np.asarray(a, dtype=np.float32)

    wbv = np.zeros((128, WB_N, 2, 256), f8)

    def put(key, Wmat):  # Wmat: [256, 256] fp32, already scaled
        e = WB_ENT[key]
        for k in range(2):
            wbv[:, e, k, :] = Wmat[k * 128:(k + 1) * 128].astype(f8)

    oW1 = f32a(inputs["o_W1"])
    oW2 = f32a(inputs["o_W2"])
    aW0 = f32a(inputs["a_W0"])
    cW0 = f32a(inputs["c_W0"])
    tW0 = f32a(inputs["t_W0"])
    for n in range(N):
        put(("oW1", n), S * oW1[n])
        put(("Wl", n), S * (oW2[n] @ aW0[:H]))
        put(("Wr", n), S * (oW2[n] @ aW0[H:]))
        put(("Wc", n), S * (oW2[n] @ cW0))
        put(("Wt", n), S * (oW2[n] @ tW0))
    put(("cW1",), S * f32a(inputs["c_W1"]))
    put(("tW1",), S * f32a(inputs["t_W1"]))
    put(("aW1",), S * f32a(inputs["a_W1"]))
    for key, src in ((("w2c",), "c_W2"), (("w2t",), "t_W2"), (("w2a",), "a_W2")):
        w2 = S * f32a(inputs[src])[:, 0]
        e = WB_ENT[key]
        for k in range(2):
            wbv[:, e, k, 128] = w2[k * 128:(k + 1) * 128].astype(f8)

    biasv = np.zeros((128, BIAS_COLS), np.float32)

    def putb(key, vec):
        biasv[:, BIAS_OFF[key]] = vec

    ob2 = f32a(inputs["o_b2"])
    blv = ob2 @ aW0[:H] + f32a(inputs["a_b0"])[None, :]
    brv = ob2 @ aW0[H:]
    bcv = ob2 @ cW0 + f32a(inputs["c_b0"])[None, :]
    btv = ob2 @ tW0 + f32a(inputs["t_b0"])[None, :]
    for n in range(N):
        for m in range(2):
            sl = slice(m * 128, (m + 1) * 128)
            putb(("b0", n, m), S * f32a(inputs["o_b0"])[n][sl])
            putb(("b1", n, m), S * S * f32a(inputs["o_b1"])[n][sl])
            putb(("bl", n, m), S ** 3 * blv[n][sl])
            putb(("br", n, m), S ** 3 * brv[n][sl])
            putb(("bc", n, m), S * bcv[n][sl])
            putb(("bt", n, m), S * btv[n][sl])
    for m in range(2):
        sl = slice(m * 128, (m + 1) * 128)
        putb(("cb1", m), S * S * f32a(inputs["c_b1"])[sl])
        putb(("tb1", m), S * S * f32a(inputs["t_b1"])[sl])
        putb(("ab1", m), S * S * f32a(inputs["a_b1"])[sl])
    finb = np.zeros(128, np.float32)
    for i in range(N):
        finb[i * 10:i * 10 + 8] = f32a(inputs["a_b2"])[0]
        finb[i * 10 + 8] = f32a(inputs["c_b2"])[0]
        finb[i * 10 + 9] = f32a(inputs["t_b2"])[0]
    putb(("finb",), finb)

    zero_bias = all(
        not np.any(f32a(inputs[k]))
        for k in ("o_b0", "o_b1", "o_b2", "c_b0", "c_b1", "t_b0", "t_b1",
                  "a_b0", "a_b1")
    )

    ow0v = np.zeros((IN, N * H), bf)
    oW0 = f32a(inputs["o_W0"])
    for n in range(N):
        ow0v[:, n * H:(n + 1) * H] = (S * oW0[n]).astype(bf)

    xT = np.ascontiguousarray(f32a(inputs["x"]).T)
    common = {"wb": wbv.reshape(128, -1), "bias": biasv}
    in_maps = []
    for c in range(NCORES):
        m = dict(common)
        xw0 = np.empty((IN, BC + N * H), bf)
        xw0[:, :BC] = xT[:, c * BC:(c + 1) * BC].astype(bf)
        xw0[:, BC:] = ow0v
        m["xw0"] = xw0
        in_maps.append(m)
    return in_maps, zero_bias


def run(inputs, trace=False, **kw):
    in_maps, zero_bias = _prep_inputs(inputs)
    key = ("nc", zero_bias)
    if key not in _CACHE:
        _CACHE[key] = _build(zero_bias)
    nc = _CACHE[key]
    res = run_bass_kernel_spmd(nc, in_maps, list(range(NCORES)), trace=trace, **kw)
    out = np.concatenate([res.results[c]["outT"].T for c in range(NCORES)], axis=0)
    return out.astype(np.float32), res


def kernel(**inputs) -> np.ndarray:
    out, _ = run(inputs, trace=False)
    return out


# revision 7
# speedup vs baseline: 3.1610x; 3.1610x over previous
"""BlockStackingSGN kernel for 8 Trainium2 NeuronCores.

Data-parallel over batch B=4096 (512 rows/core; batch in the free dim,
hidden on partitions). Key optimizations over a bf16 tiling:

- fp8e4m3 DoubleRow matmuls for every 256-deep contraction: one PE
  instruction contracts both 128-row k-tiles in the cycles of one,
  halving PE time.
- The linear object-encoder output layer (no relu) is folded on the host
  into its four downstream consumers (AonB-left/right, clear, ontable
  first layers), deleting that layer's matmuls and evacuations.
- Power-of-2 scaling (weights x16) keeps fp8 weights out of the
  subnormal range; scales flow through relu/add transparently and are
  absorbed for free by activation-engine scale or a tensor_scalar
  multiply, so every PSUM evacuation is a single instruction.
- Early phases run two 256-wide layers per 4-bank PSUM tile so one
  evacuation instruction drains four matmul accumulations (GpSimd
  cannot read PSUM, so evacuations are split across Scalar+Vector only;
  GpSimd handles the SBUF-side pair adds and relu casts).
- All 80 output heads (AonB pairs / clear / ontable) accumulate into one
  PSUM bank via one-hot fp8 stationaries sliced from a sliding window;
  a single batched Sigmoid finishes the kernel.
"""

import sys

import numpy as np

sys.path.insert(0, "/opt/trn_rl_repo")

import concourse.bacc as bacc
import concourse.mybir as mybir
import concourse.tile as tile
from concourse.bass_utils import run_bass_kernel_spmd

dt = mybir.dt
AF = mybir.ActivationFunctionType
ALU = mybir.AluOpType
PM = mybir.MatmulPerfMode

N = 8
H = 256
B = 4096
IN = 3 * N
NCORES = 8
BC = B // NCORES          # 512 batch rows per core
W = BC
R = N * (N + 2)           # 80 output rows
S = 16.0                  # weight scale 2^4

F32 = dt.float32
BF16 = dt.bfloat16
FP8 = dt.float8e4

_CACHE = {}


def _wb_layout():
    """fp8 weight tile entries of [128, 2, 256] (512 cols each), ordered by
    first use (doubles as DMA arrival order)."""
    keys = []
    for n in range(N):
        keys.append(("oW1", n))
    for n in range(N):
        keys.append(("Wl", n))
        keys.append(("Wr", n))
    for n in range(N):
        keys.append(("Wc", n))
        keys.append(("Wt", n))
    keys += [("cW1",), ("tW1",), ("w2c",), ("w2t",), ("aW1",), ("w2a",)]
    return {k: i for i, k in enumerate(keys)}, len(keys)


WB_ENT, WB_N = _wb_layout()
WB_COLS = WB_N * 512


def _bias_layout():
    keys = []
    for n in range(N):
        for nm in ("b0", "b1", "bl", "br", "bc", "bt"):
            for m in range(2):
                keys.append((nm, n, m))
    for nm in ("cb1", "tb1", "ab1"):
        for m in range(2):
            keys.append((nm, m))
    keys.append(("finb",))
    return {k: i for i, k in enumerate(keys)}, len(keys)


BIAS_OFF, BIAS_COLS = _bias_layout()

N_DMA_CHUNKS = 8


def _build(zero_bias):
    nc = bacc.Bacc("TRN2", target_bir_lowering=False, debug=False, num_devices=NCORES)

    d_xw0 = nc.dram_tensor("xw0", [IN, BC + N * H], BF16, kind="ExternalInput")
    d_wb = nc.dram_tensor("wb", [128, WB_COLS], FP8, kind="ExternalInput")
    d_bias = nc.dram_tensor("bias", [128, BIAS_COLS], F32, kind="ExternalInput")
    d_out = nc.dram_tensor("outT", [R, BC], F32, kind="ExternalOutput")

    K2 = 2 * W   # 1024: one 256-wide activation (2 k-tiles x 512 batch)

    with tile.TileContext(nc) as tc:
        with (
            tc.tile_pool(name="w", bufs=1) as wp,
            tc.tile_pool(name="act", bufs=1) as acp,
            tc.tile_pool(name="wk", bufs=4) as wk,
            tc.tile_pool(name="ph2", bufs=3) as php,
        ):
            xw0 = wp.tile([IN, BC + N * H], BF16, tag="xw0")
            nc.sync.dma_start(xw0[:], d_xw0[:])
            xT = xw0[:, :BC]
            bias = wp.tile([128, BIAS_COLS], F32, tag="bias")
            nc.gpsimd.dma_start(bias[:], d_bias[:])

            wb = wp.tile([128, WB_N, 2, 256], FP8, tag="wb")
            chunk = (WB_N + N_DMA_CHUNKS - 1) // N_DMA_CHUNKS
            for c in range(N_DMA_CHUNKS):
                eng = nc.gpsimd if c % 2 == 0 else nc.sync
                lo, hi = c * chunk, min((c + 1) * chunk, WB_N)
                if lo < hi:
                    eng.dma_start(wb[:, lo:hi], d_wb[:, lo * 512:hi * 512])

            def wsl(key, m):
                return wb[:, WB_ENT[key], :, m * 128:(m + 1) * 128]

            def w2sl(key, r):
                return wb[:, WB_ENT[key], :, 128 - r:256 - r]

            def bcol(key):
                return bias[:, BIAS_OFF[key]:BIAS_OFF[key] + 1]

            def engine(e):
                return {"A": nc.scalar, "D": nc.vector, "P": nc.gpsimd}[e]

            def asdr(ap2d):
                """view a [128, 1024] activation slice as DR rhs [128, 2, 512]"""
                return ap2d.rearrange("p (k w) -> p k w", k=2)

            def evac(e, out2, ps2, bkeys, relu, scale):
                """out = func(scale * psum + scale*bias). One instruction when
                biases are zero, else one per [128, W] column block."""
                if zero_bias:
                    if e == "A":
                        func = AF.Relu if relu else AF.Identity
                        nc.scalar.activation(out2, ps2, func, scale=scale)
                    elif relu:
                        # (mult scale, max 0): measured faster than plain max
                        engine(e).tensor_scalar(out2, ps2, scale, 0.0,
                                                ALU.mult, ALU.max)
                    else:
                        engine(e).tensor_scalar(out2, ps2, scale, None, ALU.mult)
                else:
                    nsub = out2.shape[-1] // W if len(out2.shape) == 2 else 2
                    for m in range(nsub):
                        o = out2[:, m * W:(m + 1) * W]
                        p = ps2[:, m * W:(m + 1) * W]
                        b = bcol(bkeys[m])
                        if e == "A" or (relu and scale != 1.0):
                            func = AF.Relu if relu else AF.Identity
                            nc.scalar.activation(o, p, func, bias=b, scale=scale)
                        elif relu:
                            engine(e).tensor_scalar(o, p, b, 0.0, ALU.add, ALU.max)
                        else:
                            engine(e).tensor_scalar(o, p, b, scale,
                                                    ALU.add, ALU.mult)

            cts = {}

            def pick(seq, key):
                c = cts.setdefault(key, [0])
                e = seq[c[0] % len(seq)]
                c[0] += 1
                return e

            SEQ_ERA = "AADA"      # era-A 4-bank evacs
            SEQ_Y1 = "DDDD"
            SEQ_YE = "AADA"       # pair y evacs (4-bank)
            SEQ_ADD = "DDPDP"
            # relu-cast: all DVE (736ns / 2 pairs measured)

            # ================= era A: 4-bank psum tiles =================
            h0 = acp.tile([128, N * K2], FP8, tag="h0")
            h1 = acp.tile([128, N * K2], FP8, tag="h1")
            al = acp.tile([128, N * K2], BF16, tag="al")
            ar = acp.tile([128, N * K2], BF16, tag="ar")

            def blk(t, n, q=1):
                return t[:, n * K2:(n + q) * K2]

            with tc.tile_pool(name="pa", bufs=2, space="PSUM") as pa:
                # L0 (bf16, contraction 24): two blocks per psum tile
                for n in range(0, N, 2):
                    pst = pa.tile([128, 2 * K2], F32, tag="pa", name=f"psA{n}")
                    for q in range(2):
                        for m in range(2):
                            o = BC + (n + q) * H + m * 128
                            nc.tensor.matmul(
                                pst[:, (2 * q + m) * W:(2 * q + m + 1) * W],
                                xw0[:, o:o + 128], xT, start=True, stop=True,
                                skip_group_check=True)
                    evac(pick(SEQ_ERA, "h0"), blk(h0, n, 2), pst[:],
                         [("b0", n + q, m) for q in range(2) for m in range(2)],
                         True, 1.0)
                # L1 (DR)
                for n in range(0, N, 2):
                    pst = pa.tile([128, 2 * K2], F32, tag="pa", name=f"psB{n}")
                    for q in range(2):
                        for m in range(2):
                            nc.tensor.matmul(
                                pst[:, (2 * q + m) * W:(2 * q + m + 1) * W],
                                wsl(("oW1", n + q), m), asdr(blk(h0, n + q)),
                                start=True, stop=True, perf_mode=PM.DoubleRow,
                                skip_group_check=True)
                    evac(pick(SEQ_ERA, "h1"), blk(h1, n, 2), pst[:],
                         [("b1", n + q, m) for q in range(2) for m in range(2)],
                         True, 1.0)
                # al / ar (DR, enc folded; bf16 out at 2^8)
                for n in range(0, N, 2):
                    for dst, key, bk in ((al, "Wl", "bl"), (ar, "Wr", "br")):
                        pst = pa.tile([128, 2 * K2], F32, tag="pa",
                                      name=f"psC{n}{key}")
                        for q in range(2):
                            for m in range(2):
                                nc.tensor.matmul(
                                    pst[:, (2 * q + m) * W:(2 * q + m + 1) * W],
                                    wsl((key, n + q), m), asdr(blk(h1, n + q)),
                                    start=True, stop=True, perf_mode=PM.DoubleRow,
                                    skip_group_check=True)
                        evac(pick(SEQ_ERA, "al"), blk(dst, n, 2), pst[:],
                             [(bk, n + q, m) for q in range(2) for m in range(2)],
                             False, 1.0 / S)

            # ================= era B: pair loop + preds =================
            with (
                tc.tile_pool(name="py", bufs=1, space="PSUM") as py,
                tc.tile_pool(name="pp", bufs=1, space="PSUM") as pp,
                tc.tile_pool(name="pf", bufs=1, space="PSUM") as pf,
            ):
                fin = pf.tile([128, BC], F32, tag="fin")
                n_fin = N * N + 2 * N
                fin_ct = [0]

                def fin_mm(w2key, r, rhs2):
                    first = fin_ct[0] == 0
                    fin_ct[0] += 1
                    last = fin_ct[0] == n_fin
                    nc.tensor.matmul(fin[:], w2sl(w2key, r), asdr(rhs2),
                                     start=first, stop=last,
                                     perf_mode=PM.DoubleRow)

                def dr2(ps2, key, rhs2):
                    for m in range(2):
                        nc.tensor.matmul(ps2[:, m * W:(m + 1) * W], wsl(key, m),
                                         asdr(rhs2), start=True, stop=True,
                                         perf_mode=PM.DoubleRow,
                                         skip_group_check=True)

                def pred_thunk(n, w0k, w1k, b0k, b1k, w2k, r):
                    def go():
                        y0 = wk.tile([128, K2], FP8, tag="y0",
                                     name=f"y0_{n}_{w0k}")
                        pst = pp.tile([128, K2], F32, tag="pp",
                                      name=f"psY0{n}{w0k}")
                        dr2(pst, (w0k, n), blk(h1, n))
                        evac("A", y0[:], pst[:],
                             [(b0k, n, m) for m in range(2)], True, 1.0 / (S * S))
                        y1 = wk.tile([128, K2], FP8, tag="y1",
                                     name=f"y1_{n}_{w0k}")
                        pst2 = pp.tile([128, K2], F32, tag="pp",
                                       name=f"psY1{n}{w0k}")
                        dr2(pst2, w1k, y0[:])
                        evac(pick(SEQ_Y1, "y1"), y1[:], pst2[:],
                             [(b1k, m) for m in range(2)], True, 1.0)
                        fin_mm(w2k, r, y1[:])
                    return go

                preds = []
                for n in range(N):
                    preds.append(pred_thunk(n, "Wc", ("cW1",), "bc", "cb1",
                                            ("w2c",), n * 10 + 8))
                    preds.append(pred_thunk(n, "Wt", ("tW1",), "bt", "tb1",
                                            ("w2t",), n * 10 + 9))

                # pair loop: 2-pair blocks, preds interleaved
                for i in range(N):
                    for jj in range(0, N, 2):
                        phs = php.tile([128, 2 * K2], BF16, tag="phs",
                                       name=f"phs{i}{jj}")
                        for u in range(2):
                            ea = pick(SEQ_ADD, "add")
                            engine(ea).tensor_tensor(
                                phs[:, u * K2:(u + 1) * K2], blk(al, i),
                                blk(ar, jj + u), ALU.add)
                        ph = php.tile([128, 2 * K2], FP8, tag="ph",
                                      name=f"ph{i}{jj}")
                        nc.vector.tensor_scalar(ph[:], phs[:], 1.0 / S, 0.0,
                                                ALU.mult, ALU.max)
                        if jj % 4 == 0 and preds:
                            preds.pop(0)()
                        pst = py.tile([128, 2 * K2], F32, tag="py",
                                      name=f"psP{i}{jj}")
                        for u in range(2):
                            for m in range(2):
                                nc.tensor.matmul(
                                    pst[:, (2 * u + m) * W:(2 * u + m + 1) * W],
                                    wsl(("aW1",), m),
                                    asdr(ph[:, u * K2:(u + 1) * K2]),
                                    start=True, stop=True, perf_mode=PM.DoubleRow,
                                    skip_group_check=True)
                        y2 = wk.tile([128, 2 * K2], FP8, tag="y2",
                                     name=f"y2_{i}{jj}")
                        evac(pick(SEQ_YE, "ye"), y2[:], pst[:],
                             [("ab1", m) for m in range(2)] * 2, True, 1.0)
                        for u in range(2):
                            fin_mm(("w2a",), i * 10 + jj + u,
                                   y2[:, u * K2:(u + 1) * K2])
                for t in preds:
                    t()
                assert fin_ct[0] == n_fin

                # batched sigmoid + store
                outT = wk.tile([128, BC], F32, tag="outT")
                nc.scalar.activation(outT[:], fin[:], AF.Sigmoid,
                                     bias=bcol(("finb",)), scale=1.0 / (S ** 3))
                nc.sync.dma_start(d_out[:], outT[:R, :])

    nc.compile()
    return nc


def _prep_inputs(inputs):
    import ml_dtypes

    bf = ml_dtypes.bfloat16
    f8 = ml_dtypes.float8_e4m3fn
    f32a = lambda a: np.asarray(a, dtype=np.float32)

    wbv = np.zeros((128, WB_N, 2, 256), f8)

    def put(key, Wmat):  # Wmat: [256, 256] fp32, already scaled
        e = WB_ENT[key]
        for k in range(2):
            wbv[:, e, k, :] = Wmat[k * 128:(k + 1) * 128].astype(f8)

    oW1 = f32a(inputs["o_W1"])
    oW2 = f32a(inputs["o_W2"])
    aW0 = f32a(inputs["a_W0"])
    cW0 = f32a(inputs["c_W0"])
    tW0 = f32a(inputs["t_W0"])
    for n in range(N):
        put(("oW1", n), S * oW1[n])
        put(("Wl", n), S * (oW2[n] @ aW0[:H]))
        put(("Wr", n), S * (oW2[n] @ aW0[H:]))
        put(("Wc", n), S * (oW2[n] @ cW0))
        put(("Wt", n), S * (oW2[n] @ tW0))
    put(("cW1",), S * f32a(inputs["c_W1"]))
    put(("tW1",), S * f32a(inputs["t_W1"]))
    put(("aW1",), S * f32a(inputs["a_W1"]))
    for key, src in ((("w2c",), "c_W2"), (("w2t",), "t_W2"), (("w2a",), "a_W2")):
        w2 = S * f32a(inputs[src])[:, 0]
        e = WB_ENT[key]
        for k in range(2):
            wbv[:, e, k, 128] = w2[k * 128:(k + 1) * 128].astype(f8)

    biasv = np.zeros((128, BIAS_COLS), np.float32)

    def putb(key, vec):
        biasv[:, BIAS_OFF[key]] = vec

    ob2 = f32a(inputs["o_b2"])
    blv = ob2 @ aW0[:H] + f32a(inputs["a_b0"])[None, :]
    brv = ob2 @ aW0[H:]
    bcv = ob2 @ cW0 + f32a(inputs["c_b0"])[None, :]
    btv = ob2 @ tW0 + f32a(inputs["t_b0"])[None, :]
    for n in range(N):
        for m in range(2):
            sl = slice(m * 128, (m + 1) * 128)
            putb(("b0", n, m), S * f32a(inputs["o_b0"])[n][sl])
            putb(("b1", n, m), S * S * f32a(inputs["o_b1"])[n][sl])
            putb(("bl", n, m), S ** 3 * blv[n][sl])
            putb(("br", n, m), S ** 3 * brv[n][sl])
            putb(("bc", n, m), S * bcv[n][sl])
            putb(("bt", n, m), S * btv[n][sl])
    for m in range(2):
        sl = slice(m * 128, (m + 1) * 128)
        putb(("cb1", m), S * S * f32a(inputs["c_b1"])[sl])
        putb(("tb1", m), S * S * f32a(inputs["t_b1"])[sl])
        putb(("ab1", m), S * S * f32a(inputs["a_b1"])[sl])
    finb = np.zeros(128, np.float32)
    for i in range(N):
        finb[i * 10:i * 10 + 8] = f32a(inputs["a_b2"])[0]
        finb[i * 10 + 8] = f32a(inputs["c_b2"])[0]
        finb[i * 10 + 9] = f32a(inputs["t_b2"])[0]
    putb(("finb",), finb)

    zero_bias = all(
        not np.any(f32a(inputs[k]))
        for k in ("o_b0", "o_b1", "o_b2", "c_b0", "c_b1", "t_b0", "t_b1",
                  "a_b0", "a_b1")
    )

    ow0v = np.zeros((IN, N * H), bf)
    oW0 = f32a(inputs["o_W0"])
    for n in range(N):
        ow0v[:, n * H:(n + 1) * H] = (S * oW0[n]).astype(bf)

    xT = np.ascontiguousarray(f32a(inputs["x"]).T)
    common = {"wb": wbv.reshape(128, -1), "bias": biasv}
    in_maps = []
    for c in range(NCORES):
        m = dict(common)
        xw0 = np.empty((IN, BC + N * H), bf)
        xw0[:, :BC] = xT[:, c * BC:(c + 1) * BC].astype(bf)
        xw0[:, BC:] = ow0v
        m["xw0"] = xw0
        in_maps.append(m)
    return in_maps, zero_bias


def run(inputs, trace=False, **kw):
    in_maps, zero_bias = _prep_inputs(inputs)
    key = ("nc", zero_bias)
    if key not in _CACHE:
        _CACHE[key] = _build(zero_bias)
    nc = _CACHE[key]
    res = run_bass_kernel_spmd(nc, in_maps, list(range(NCORES)), trace=trace, **kw)
    out = np.concatenate([res.results[c]["outT"].T for c in range(NCORES)], axis=0)
    return out.astype(np.float32), res


def kernel(**inputs) -> np.ndarray:
    out, _ = run(inputs, trace=False)
    return out


# revision 8
# speedup vs baseline: 3.1808x; 1.0063x over previous
"""BlockStackingSGN kernel for 8 Trainium2 NeuronCores.

Data-parallel over batch B=4096 (512 rows/core; batch in the free dim,
hidden on partitions). Key optimizations over a bf16 tiling:

- fp8e4m3 DoubleRow matmuls for every 256-deep contraction: one PE
  instruction contracts both 128-row k-tiles in the cycles of one,
  halving PE time.
- The linear object-encoder output layer (no relu) is folded on the host
  into its four downstream consumers (AonB-left/right, clear, ontable
  first layers), deleting that layer's matmuls and evacuations.
- Power-of-2 scaling (weights x16) keeps fp8 weights out of the
  subnormal range; scales flow through relu/add transparently and are
  absorbed for free by activation-engine scale or a tensor_scalar
  multiply, so every PSUM evacuation is a single instruction.
- Early phases run two 256-wide layers per 4-bank PSUM tile so one
  evacuation instruction drains four matmul accumulations (GpSimd
  cannot read PSUM, so evacuations are split across Scalar+Vector only;
  GpSimd handles the SBUF-side pair adds and relu casts).
- All 80 output heads (AonB pairs / clear / ontable) accumulate into one
  PSUM bank via one-hot fp8 stationaries sliced from a sliding window;
  a single batched Sigmoid finishes the kernel.
"""

import sys

import numpy as np

sys.path.insert(0, "/opt/trn_rl_repo")

import concourse.bacc as bacc
import concourse.mybir as mybir
import concourse.tile as tile
from concourse.bass_utils import run_bass_kernel_spmd

dt = mybir.dt
AF = mybir.ActivationFunctionType
ALU = mybir.AluOpType
PM = mybir.MatmulPerfMode

N = 8
H = 256
B = 4096
IN = 3 * N
NCORES = 8
BC = B // NCORES          # 512 batch rows per core
W = BC
R = N * (N + 2)           # 80 output rows
S = 16.0                  # weight scale 2^4

F32 = dt.float32
BF16 = dt.bfloat16
FP8 = dt.float8e4

_CACHE = {}


def _wb_layout():
    """fp8 weight tile entries of [128, 2, 256] (512 cols each), ordered by
    first use (doubles as DMA arrival order)."""
    keys = []
    for n in range(N):
        keys.append(("oW1", n))
    for n in range(N):
        keys.append(("Wl", n))
        keys.append(("Wr", n))
    for n in range(N):
        keys.append(("Wc", n))
        keys.append(("Wt", n))
    keys += [("cW1",), ("tW1",), ("w2c",), ("w2t",), ("aW1",), ("w2a",)]
    return {k: i for i, k in enumerate(keys)}, len(keys)


WB_ENT, WB_N = _wb_layout()
WB_COLS = WB_N * 512


def _bias_layout():
    keys = []
    for n in range(N):
        for nm in ("b0", "b1", "bl", "br", "bc", "bt"):
            for m in range(2):
                keys.append((nm, n, m))
    for nm in ("cb1", "tb1", "ab1"):
        for m in range(2):
            keys.append((nm, m))
    keys.append(("finb",))
    return {k: i for i, k in enumerate(keys)}, len(keys)


BIAS_OFF, BIAS_COLS = _bias_layout()

N_DMA_CHUNKS = 8


def _build(zero_bias):
    nc = bacc.Bacc("TRN2", target_bir_lowering=False, debug=False, num_devices=NCORES)

    d_xw0 = nc.dram_tensor("xw0", [IN, BC + N * H], BF16, kind="ExternalInput")
    d_wb = nc.dram_tensor("wb", [128, WB_COLS], FP8, kind="ExternalInput")
    d_bias = nc.dram_tensor("bias", [128, BIAS_COLS], F32, kind="ExternalInput")
    d_out = nc.dram_tensor("outT", [R, BC], F32, kind="ExternalOutput")

    K2 = 2 * W   # 1024: one 256-wide activation (2 k-tiles x 512 batch)

    with tile.TileContext(nc) as tc:
        with (
            tc.tile_pool(name="w", bufs=1) as wp,
            tc.tile_pool(name="act", bufs=1) as acp,
            tc.tile_pool(name="wk", bufs=4) as wk,
            tc.tile_pool(name="ph2", bufs=10) as php,
        ):
            xw0 = wp.tile([IN, BC + N * H], BF16, tag="xw0")
            nc.sync.dma_start(xw0[:], d_xw0[:])
            xT = xw0[:, :BC]
            bias = wp.tile([128, BIAS_COLS], F32, tag="bias")
            nc.gpsimd.dma_start(bias[:], d_bias[:])

            wb = wp.tile([128, WB_N, 2, 256], FP8, tag="wb")
            chunk = (WB_N + N_DMA_CHUNKS - 1) // N_DMA_CHUNKS
            for c in range(N_DMA_CHUNKS):
                eng = nc.gpsimd if c % 2 == 0 else nc.sync
                lo, hi = c * chunk, min((c + 1) * chunk, WB_N)
                if lo < hi:
                    eng.dma_start(wb[:, lo:hi], d_wb[:, lo * 512:hi * 512])

            def wsl(key, m):
                return wb[:, WB_ENT[key], :, m * 128:(m + 1) * 128]

            def w2sl(key, r):
                return wb[:, WB_ENT[key], :, 128 - r:256 - r]

            def bcol(key):
                return bias[:, BIAS_OFF[key]:BIAS_OFF[key] + 1]

            def engine(e):
                return {"A": nc.scalar, "D": nc.vector, "P": nc.gpsimd}[e]

            def asdr(ap2d):
                """view a [128, 1024] activation slice as DR rhs [128, 2, 512]"""
                return ap2d.rearrange("p (k w) -> p k w", k=2)

            def evac(e, out2, ps2, bkeys, relu, scale):
                """out = func(scale * psum + scale*bias). One instruction when
                biases are zero, else one per [128, W] column block."""
                if zero_bias:
                    if e == "A":
                        func = AF.Relu if relu else AF.Identity
                        nc.scalar.activation(out2, ps2, func, scale=scale)
                    elif relu:
                        # (mult scale, max 0): measured faster than plain max
                        engine(e).tensor_scalar(out2, ps2, scale, 0.0,
                                                ALU.mult, ALU.max)
                    else:
                        engine(e).tensor_scalar(out2, ps2, scale, None, ALU.mult)
                else:
                    nsub = out2.shape[-1] // W if len(out2.shape) == 2 else 2
                    for m in range(nsub):
                        o = out2[:, m * W:(m + 1) * W]
                        p = ps2[:, m * W:(m + 1) * W]
                        b = bcol(bkeys[m])
                        if e == "A" or (relu and scale != 1.0):
                            func = AF.Relu if relu else AF.Identity
                            nc.scalar.activation(o, p, func, bias=b, scale=scale)
                        elif relu:
                            engine(e).tensor_scalar(o, p, b, 0.0, ALU.add, ALU.max)
                        else:
                            engine(e).tensor_scalar(o, p, b, scale,
                                                    ALU.add, ALU.mult)

            cts = {}

            def pick(seq, key):
                c = cts.setdefault(key, [0])
                e = seq[c[0] % len(seq)]
                c[0] += 1
                return e

            SEQ_ERA = "AAAD"      # era-A 4-bank evacs
            SEQ_Y1 = "AAD"
            SEQ_YE = "AAAD"       # pair y evacs
            SEQ_ADD = "PD"
            # relu-cast: all DVE

            # ================= era A: 4-bank psum tiles =================
            h0 = acp.tile([128, N * K2], FP8, tag="h0")
            h1 = acp.tile([128, N * K2], FP8, tag="h1")
            al = acp.tile([128, N * K2], BF16, tag="al")
            ar = acp.tile([128, N * K2], BF16, tag="ar")

            def blk(t, n, q=1):
                return t[:, n * K2:(n + q) * K2]

            with tc.tile_pool(name="pa", bufs=2, space="PSUM") as pa:
                # L0 (bf16, contraction 24): two blocks per psum tile
                for n in range(0, N, 2):
                    pst = pa.tile([128, 2 * K2], F32, tag="pa", name=f"psA{n}")
                    for q in range(2):
                        for m in range(2):
                            o = BC + (n + q) * H + m * 128
                            nc.tensor.matmul(
                                pst[:, (2 * q + m) * W:(2 * q + m + 1) * W],
                                xw0[:, o:o + 128], xT, start=True, stop=True,
                                skip_group_check=True)
                    evac(pick(SEQ_ERA, "h0"), blk(h0, n, 2), pst[:],
                         [("b0", n + q, m) for q in range(2) for m in range(2)],
                         True, 1.0)
                # L1 (DR)
                for n in range(0, N, 2):
                    pst = pa.tile([128, 2 * K2], F32, tag="pa", name=f"psB{n}")
                    for q in range(2):
                        for m in range(2):
                            nc.tensor.matmul(
                                pst[:, (2 * q + m) * W:(2 * q + m + 1) * W],
                                wsl(("oW1", n + q), m), asdr(blk(h0, n + q)),
                                start=True, stop=True, perf_mode=PM.DoubleRow,
                                skip_group_check=True)
                    evac(pick(SEQ_ERA, "h1"), blk(h1, n, 2), pst[:],
                         [("b1", n + q, m) for q in range(2) for m in range(2)],
                         True, 1.0)
                # al / ar (DR, enc folded; bf16 out at 2^8)
                for n in range(0, N, 2):
                    for dst, key, bk in ((al, "Wl", "bl"), (ar, "Wr", "br")):
                        pst = pa.tile([128, 2 * K2], F32, tag="pa",
                                      name=f"psC{n}{key}")
                        for q in range(2):
                            for m in range(2):
                                nc.tensor.matmul(
                                    pst[:, (2 * q + m) * W:(2 * q + m + 1) * W],
                                    wsl((key, n + q), m), asdr(blk(h1, n + q)),
                                    start=True, stop=True, perf_mode=PM.DoubleRow,
                                    skip_group_check=True)
                        evac(pick(SEQ_ERA, "al"), blk(dst, n, 2), pst[:],
                             [(bk, n + q, m) for q in range(2) for m in range(2)],
                             False, 1.0 / S)

            # ================= era B: pair loop + preds =================
            with (
                tc.tile_pool(name="py", bufs=2, space="PSUM") as py,
                tc.tile_pool(name="pp", bufs=1, space="PSUM") as pp,
                tc.tile_pool(name="pf", bufs=1, space="PSUM") as pf,
            ):
                fin = pf.tile([128, BC], F32, tag="fin")
                n_fin = N * N + 2 * N
                fin_ct = [0]

                def fin_mm(w2key, r, rhs2):
                    first = fin_ct[0] == 0
                    fin_ct[0] += 1
                    last = fin_ct[0] == n_fin
                    nc.tensor.matmul(fin[:], w2sl(w2key, r), asdr(rhs2),
                                     start=first, stop=last,
                                     perf_mode=PM.DoubleRow)

                def dr2(ps2, key, rhs2):
                    for m in range(2):
                        nc.tensor.matmul(ps2[:, m * W:(m + 1) * W], wsl(key, m),
                                         asdr(rhs2), start=True, stop=True,
                                         perf_mode=PM.DoubleRow,
                                         skip_group_check=True)

                def pred_thunk(n, w0k, w1k, b0k, b1k, w2k, r):
                    def go():
                        y0 = wk.tile([128, K2], FP8, tag="y0",
                                     name=f"y0_{n}_{w0k}")
                        pst = pp.tile([128, K2], F32, tag="pp",
                                      name=f"psY0{n}{w0k}")
                        dr2(pst, (w0k, n), blk(h1, n))
                        evac("A", y0[:], pst[:],
                             [(b0k, n, m) for m in range(2)], True, 1.0 / (S * S))
                        y1 = wk.tile([128, K2], FP8, tag="y1",
                                     name=f"y1_{n}_{w0k}")
                        pst2 = pp.tile([128, K2], F32, tag="pp",
                                       name=f"psY1{n}{w0k}")
                        dr2(pst2, w1k, y0[:])
                        evac(pick(SEQ_Y1, "y1"), y1[:], pst2[:],
                             [(b1k, m) for m in range(2)], True, 1.0)
                        fin_mm(w2k, r, y1[:])
                    return go

                preds = []
                for n in range(N):
                    preds.append(pred_thunk(n, "Wc", ("cW1",), "bc", "cb1",
                                            ("w2c",), n * 10 + 8))
                    preds.append(pred_thunk(n, "Wt", ("tW1",), "bt", "tb1",
                                            ("w2t",), n * 10 + 9))

                # pair loop, software-pipelined: ph production (DVE/Pool)
                # runs one i-group ahead of consumption (PE + evacs)
                PH = {}

                def emit_prod(i):
                    for jj in range(0, N, 2):
                        phs = php.tile([128, 2 * K2], BF16, tag="phs",
                                       name=f"phs{i}{jj}")
                        for u in range(2):
                            ea = pick(SEQ_ADD, "add")
                            engine(ea).tensor_tensor(
                                phs[:, u * K2:(u + 1) * K2], blk(al, i),
                                blk(ar, jj + u), ALU.add)
                        ph = php.tile([128, 2 * K2], FP8, tag="ph",
                                      name=f"ph{i}{jj}")
                        nc.vector.tensor_scalar(ph[:], phs[:], 1.0 / S, 0.0,
                                                ALU.mult, ALU.max)
                        PH[(i, jj)] = ph

                def emit_cons(i):
                    for jj in range(0, N, 2):
                        ph = PH.pop((i, jj))
                        if jj % 4 == 0 and preds:
                            preds.pop(0)()
                        for u in range(2):
                            j = jj + u
                            pst = py.tile([128, K2], F32, tag="py",
                                          name=f"psP{i}{j}")
                            for m in range(2):
                                nc.tensor.matmul(
                                    pst[:, m * W:(m + 1) * W], wsl(("aW1",), m),
                                    asdr(ph[:, u * K2:(u + 1) * K2]),
                                    start=True, stop=True, perf_mode=PM.DoubleRow,
                                    skip_group_check=True)
                            y = wk.tile([128, K2], FP8, tag="y",
                                        name=f"y_{i}{j}")
                            evac(pick(SEQ_YE, "ye"), y[:], pst[:],
                                 [("ab1", m) for m in range(2)], True, 1.0)
                            fin_mm(("w2a",), i * 10 + j, y[:])

                emit_prod(0)
                for i in range(N):
                    if i + 1 < N:
                        emit_prod(i + 1)
                    emit_cons(i)
                for t in preds:
                    t()
                assert fin_ct[0] == n_fin

                # batched sigmoid + store
                outT = wk.tile([128, BC], F32, tag="outT")
                nc.scalar.activation(outT[:], fin[:], AF.Sigmoid,
                                     bias=bcol(("finb",)), scale=1.0 / (S ** 3))
                nc.sync.dma_start(d_out[:], outT[:R, :])

    nc.compile()
    return nc


def _prep_inputs(inputs):
    import ml_dtypes

    bf = ml_dtypes.bfloat16
    f8 = ml_dtypes.float8_e4m3fn
    f32a = lambda a: np.asarray(a, dtype=np.float32)

    wbv = np.zeros((128, WB_N, 2, 256), f8)

    def put(key, Wmat):  # Wmat: [256, 256] fp32, already scaled
        e = WB_ENT[key]
        for k in range(2):
            wbv[:, e, k, :] = Wmat[k * 128:(k + 1) * 128].astype(f8)

    oW1 = f32a(inputs["o_W1"])
    oW2 = f32a(inputs["o_W2"])
    aW0 = f32a(inputs["a_W0"])
    cW0 = f32a(inputs["c_W0"])
    tW0 = f32a(inputs["t_W0"])
    for n in range(N):
        put(("oW1", n), S * oW1[n])
        put(("Wl", n), S * (oW2[n] @ aW0[:H]))
        put(("Wr", n), S * (oW2[n] @ aW0[H:]))
        put(("Wc", n), S * (oW2[n] @ cW0))
        put(("Wt", n), S * (oW2[n] @ tW0))
    put(("cW1",), S * f32a(inputs["c_W1"]))
    put(("tW1",), S * f32a(inputs["t_W1"]))
    put(("aW1",), S * f32a(inputs["a_W1"]))
    for key, src in ((("w2c",), "c_W2"), (("w2t",), "t_W2"), (("w2a",), "a_W2")):
        w2 = S * f32a(inputs[src])[:, 0]
        e = WB_ENT[key]
        for k in range(2):
            wbv[:, e, k, 128] = w2[k * 128:(k + 1) * 128].astype(f8)

    biasv = np.zeros((128, BIAS_COLS), np.float32)

    def putb(key, vec):
        biasv[:, BIAS_OFF[key]] = vec

    ob2 = f32a(inputs["o_b2"])
    blv = ob2 @ aW0[:H] + f32a(inputs["a_b0"])[None, :]
    brv = ob2 @ aW0[H:]
    bcv = ob2 @ cW0 + f32a(inputs["c_b0"])[None, :]
    btv = ob2 @ tW0 + f32a(inputs["t_b0"])[None, :]
    for n in range(N):
        for m in range(2):
            sl = slice(m * 128, (m + 1) * 128)
            putb(("b0", n, m), S * f32a(inputs["o_b0"])[n][sl])
            putb(("b1", n, m), S * S * f32a(inputs["o_b1"])[n][sl])
            putb(("bl", n, m), S ** 3 * blv[n][sl])
            putb(("br", n, m), S ** 3 * brv[n][sl])
            putb(("bc", n, m), S * bcv[n][sl])
            putb(("bt", n, m), S * btv[n][sl])
    for m in range(2):
        sl = slice(m * 128, (m + 1) * 128)
        putb(("cb1", m), S * S * f32a(inputs["c_b1"])[sl])
        putb(("tb1", m), S * S * f32a(inputs["t_b1"])[sl])
        putb(("ab1", m), S * S * f32a(inputs["a_b1"])[sl])
    finb = np.zeros(128, np.float32)
    for i in range(N):
        finb[i * 10:i * 10 + 8] = f32a(inputs["a_b2"])[0]
        finb[i * 10 + 8] = f32a(inputs["c_b2"])[0]
        finb[i * 10 + 9] = f32a(inputs["t_b2"])[0]
    putb(("finb",), finb)

    zero_bias = all(
        not np.any(f32a(inputs[k]))
        for k in ("o_b0", "o_b1", "o_b2", "c_b0", "c_b1", "t_b0", "t_b1",
                  "a_b0", "a_b1")
    )

    ow0v = np.zeros((IN, N * H), bf)
    oW0 = f32a(inputs["o_W0"])
    for n in range(N):
        ow0v[:, n * H:(n + 1) * H] = (S * oW0[n]).astype(bf)

    xT = np.ascontiguousarray(f32a(inputs["x"]).T)
    common = {"wb": wbv.reshape(128, -1), "bias": biasv}
    in_maps = []
    for c in range(NCORES):
        m = dict(common)
        xw0 = np.empty((IN, BC + N * H), bf)
        xw0[:, :BC] = xT[:, c * BC:(c + 1) * BC].astype(bf)
        xw0[:, BC:] = ow0v
        m["xw0"] = xw0
        in_maps.append(m)
    return in_maps, zero_bias


def run(inputs, trace=False, **kw):
    in_maps, zero_bias = _prep_inputs(inputs)
    key = ("nc", zero_bias)
    if key not in _CACHE:
        _CACHE[key] = _build(zero_bias)
    nc = _CACHE[key]
    res = run_bass_kernel_spmd(nc, in_maps, list(range(NCORES)), trace=trace, **kw)
    out = np.concatenate([res.results[c]["outT"].T for c in range(NCORES)], axis=0)
    return out.astype(np.float32), res


def kernel(**inputs) -> np.ndarray:
    out, _ = run(inputs, trace=False)
    return out
